# revision 5
# baseline (speedup 1.0000x reference)
"""BiGRU(2-layer, bidirectional, H=512) + classifier(K=31) + linear-chain CRF
loss on 8 Trainium2 NeuronCores.

Sharding: direction x batch-quarter split. Cores 0-3 run the FORWARD scans for
batch quarters 0-3 (16 batches each); cores 4-7 run the BACKWARD scans for the
same quarters. Time reversal is absorbed entirely into per-core host-prepared
inputs (reversed ids/labels, transposed CRF transitions, swapped start/end,
own-first weight row permutations), so the device program is identical on
every core (SPMD). The layer-boundary f<->b hidden-sequence exchange is a
pair-grouped ReduceScatter with mask-selected contributions. Each pair
redundantly computes classifier+CRF for its quarter; host consumes the
forward cores' outputs.

Scan structure per step (option "weights-stationary"): feature-major hidden
state hT [128p x 4k x 16b] bf16; 48 self-loading bf16 matmuls accumulate
gh = W_hh.T-tiles @ hT into PSUM [128 x 12 x 16]; gates computed feature-major
on DVE/ACT in fp32; h' written back feature-major (no transposes).

CRF forward DP runs in probability space: A_t = (E^T A_{t-1}) * exp(em_t),
with E = exp(trans) precomputed on host, periodic constant rescale by 2^-40
every 8 steps (exactly compensated in the final log), numerator via one-hot
(iota + is_equal) contractions.

Assumes labels >= 0 (mask all-true), which holds for this problem's inputs.
"""

import sys

sys.path.insert(0, "/opt/trn_rl_repo")

import math
from contextlib import ExitStack

import numpy as np

import concourse.bass as bass
import concourse.mybir as mybir
import concourse.tile as tile
from concourse import bacc
from concourse.bass_utils import run_bass_kernel_spmd

AF = mybir.ActivationFunctionType
ALU = mybir.AluOpType
F32 = mybir.dt.float32
BF16 = mybir.dt.bfloat16
I32 = mybir.dt.int32

B, V, E, H, K = 64, 21128, 256, 512, 31
NC_ = 8
BC = 16  # batches per core
M = 12  # gh feature tiles (3H/128)
KH = 4  # hidden k-tiles (H/128)
K0 = 2  # layer-0 input k-tiles (E/128)
K1 = 8  # layer-1 input k-tiles (2H/128)
RESCALE_EVERY = 8
RESCALE_SHIFT = 40.0  # multiply A by 2^-40


def build_nc(T=512, TC=32):
    nc = bacc.Bacc()
    NT = T * BC

    # ---------------- DRAM tensors ----------------
    ids_flat = nc.dram_tensor("ids_flat", [NT], I32, kind="ExternalInput")
    labs_flat = nc.dram_tensor("labs_flat", [NT], I32, kind="ExternalInput")
    emb = nc.dram_tensor("emb", [V, E], BF16, kind="ExternalInput")
    w_ihT0 = nc.dram_tensor("w_ihT0", [E, 3 * H], BF16, kind="ExternalInput")
    w_hhT0 = nc.dram_tensor("w_hhT0", [H, 3 * H], BF16, kind="ExternalInput")
    w_ihT1 = nc.dram_tensor("w_ihT1", [2 * H, 3 * H], BF16, kind="ExternalInput")
    w_hhT1 = nc.dram_tensor("w_hhT1", [H, 3 * H], BF16, kind="ExternalInput")
    bias0 = nc.dram_tensor("bias0", [M, 128], F32, kind="ExternalInput")
    bhn0 = nc.dram_tensor("bhn0", [KH, 128], F32, kind="ExternalInput")
    bias1 = nc.dram_tensor("bias1", [M, 128], F32, kind="ExternalInput")
    bhn1 = nc.dram_tensor("bhn1", [KH, 128], F32, kind="ExternalInput")
    cls_wT = nc.dram_tensor("cls_wT", [2 * H, K], BF16, kind="ExternalInput")
    cls_b = nc.dram_tensor("cls_b", [K], F32, kind="ExternalInput")
    e_trans = nc.dram_tensor("e_trans", [K, K], F32, kind="ExternalInput")
    transm = nc.dram_tensor("transm", [K, K], F32, kind="ExternalInput")
    start_raw = nc.dram_tensor("start_raw", [K], F32, kind="ExternalInput")
    end_raw = nc.dram_tensor("end_raw", [K], F32, kind="ExternalInput")
    exp_start = nc.dram_tensor("exp_start", [K], F32, kind="ExternalInput")
    exp_end = nc.dram_tensor("exp_end", [K], F32, kind="ExternalInput")
    masks = nc.dram_tensor("masks", [2], F32, kind="ExternalInput")

    emis_out = nc.dram_tensor("emis", [K, T * BC], F32, kind="ExternalOutput")
    losspart = nc.dram_tensor("losspart", [1, 1], F32, kind="ExternalOutput")

    h0_own = nc.dram_tensor("h0_own", [T, KH, 128, BC], BF16)
    h1_own = nc.dram_tensor("h1_own", [T, KH, 128, BC], BF16)
    contrib0 = nc.dram_tensor("contrib0", [2, T, KH, 128, BC], BF16)
    contrib1 = nc.dram_tensor("contrib1", [2, T, KH, 128, BC], BF16)
    rs0 = nc.dram_tensor("rs0", [T, KH, 128, BC], BF16)
    rs1 = nc.dram_tensor("rs1", [T, KH, 128, BC], BF16)

    NCH = T // TC  # scan chunks
    TOK = TC * BC  # tokens per scan chunk
    groups = [[q, 4 + q] for q in range(4)]

    with tile.TileContext(nc) as tc:
        with ExitStack() as ctx0:
            pw = ctx0.enter_context(tc.tile_pool(name="pw", bufs=1))

            # ---- persistent loads ----
            w0 = pw.tile([128, K0, 3 * H], BF16)
            nc.sync.dma_start(w0, w_ihT0[:, :].rearrange("(k p) n -> p k n", p=128))
            wh0 = pw.tile([128, KH, 3 * H], BF16)
            nc.sync.dma_start(wh0, w_hhT0[:, :].rearrange("(k p) n -> p k n", p=128))
            w1 = pw.tile([128, K1, 3 * H], BF16)
            nc.sync.dma_start(w1, w_ihT1[:, :].rearrange("(k p) n -> p k n", p=128))
            wh1 = pw.tile([128, KH, 3 * H], BF16)
            nc.sync.dma_start(wh1, w_hhT1[:, :].rearrange("(k p) n -> p k n", p=128))
            cw = pw.tile([128, K1, K], BF16)
            nc.sync.dma_start(cw, cls_wT[:, :].rearrange("(k p) n -> p k n", p=128))
            b0_sb = pw.tile([128, M], F32)
            nc.sync.dma_start(b0_sb, bias0[:, :].rearrange("m p -> p m"))
            bh0_sb = pw.tile([128, KH], F32)
            nc.sync.dma_start(bh0_sb, bhn0[:, :].rearrange("m p -> p m"))
            b1_sb = pw.tile([128, M], F32)
            nc.sync.dma_start(b1_sb, bias1[:, :].rearrange("m p -> p m"))
            bh1_sb = pw.tile([128, KH], F32)
            nc.sync.dma_start(bh1_sb, bhn1[:, :].rearrange("m p -> p m"))
            clsb_sb = pw.tile([K, 1], F32)
            nc.sync.dma_start(clsb_sb, cls_b[:, None])
            et_sb = pw.tile([K, K], F32)
            nc.sync.dma_start(et_sb, e_trans[:, :])
            tm_sb = pw.tile([K, K], F32)
            nc.sync.dma_start(tm_sb, transm[:, :])
            sr_sb = pw.tile([K, 1], F32)
            nc.sync.dma_start(sr_sb, start_raw[:, None])
            er_sb = pw.tile([K, 1], F32)
            nc.sync.dma_start(er_sb, end_raw[:, None])
            es_sb = pw.tile([K, 1], F32)
            nc.sync.dma_start(es_sb, exp_start[:, None])
            ee_sb = pw.tile([K, 1], F32)
            nc.sync.dma_start(ee_sb, exp_end[:, None])
            mc = pw.tile([128, 2], F32)
            nc.gpsimd.dma_start(mc, bass.AP(tensor=masks, offset=0, ap=[[0, 128], [1, 2]]))
            iota_k = pw.tile([K, 1], I32)
            nc.gpsimd.iota(iota_k, [[0, 1]], channel_multiplier=1)
            iota_f = pw.tile([K, 1], F32)
            nc.vector.tensor_copy(iota_f, iota_k)
            zero_h = pw.tile([128, KH, BC], BF16)
            nc.vector.memset(zero_h, 0.0)
            ones_k = pw.tile([K, 1], F32)
            nc.vector.memset(ones_k, 1.0)
            emisT = pw.tile([K, NT], F32)
            EM = pw.tile([K, NT], F32)

            # ================= scan layers =================
            def scan_layer(layer):
                wih = w0 if layer == 0 else w1
                whh = wh0 if layer == 0 else wh1
                bxp = b0_sb if layer == 0 else b1_sb
                bhn_sb = bh0_sb if layer == 0 else bh1_sb
                nk = K0 if layer == 0 else K1
                hseq = h0_own if layer == 0 else h1_own

                with ExitStack() as ctx:
                    px = ctx.enter_context(tc.tile_pool(name=f"px{layer}", bufs=2))
                    pxp = ctx.enter_context(tc.tile_pool(name=f"pxp{layer}", bufs=2))
                    pst = ctx.enter_context(tc.tile_pool(name=f"pst{layer}", bufs=3))
                    pg = ctx.enter_context(tc.tile_pool(name=f"pg{layer}", bufs=3))
                    pps = ctx.enter_context(
                        tc.tile_pool(name=f"pps{layer}", bufs=2, space="PSUM")
                    )
                    ppx = ctx.enter_context(
                        tc.tile_pool(name=f"ppx{layer}", bufs=2, space="PSUM")
                    )

                    h_prev = zero_h
                    for ci in range(NCH):
                        xT = px.tile([128, nk, TOK], BF16, tag="xT")
                        if layer == 0:
                            # gather embedding rows + transpose to feature-major
                            for g in range(TOK // 128):
                                ids_sb = pg.tile([128, 1], I32, tag="ids")
                                nc.gpsimd.dma_start(
                                    ids_sb,
                                    ids_flat[ci * TOK + g * 128 : ci * TOK + (g + 1) * 128][
                                        :, None
                                    ],
                                )
                                xg = pg.tile([128, E], BF16, tag="xg")
                                nc.gpsimd.indirect_dma_start(
                                    out=xg,
                                    out_offset=None,
                                    in_=emb[:, :],
                                    in_offset=bass.IndirectOffsetOnAxis(
                                        ap=ids_sb[:, :1], axis=0
                                    ),
                                )
                                for k in range(K0):
                                    nc.sync.dma_start_transpose(
                                        xT[:, k, g * 128 : (g + 1) * 128],
                                        xg[:, k * 128 : (k + 1) * 128],
                                    )
                        else:
                            # own half ascending, partner half reversed local order
                            st = KH * 128 * BC  # t-stride in elements
                            for k in range(KH):
                                nc.sync.dma_start(
                                    xT[:, k, :],
                                    bass.AP(
                                        tensor=h0_own,
                                        offset=ci * TC * st + k * 128 * BC,
                                        ap=[[BC, 128], [st, TC], [1, BC]],
                                    ),
                                )
                                nc.sync.dma_start(
                                    xT[:, KH + k, :],
                                    bass.AP(
                                        tensor=rs0,
                                        offset=(T - 1 - ci * TC) * st + k * 128 * BC,
                                        ap=[[BC, 128], [-st, TC], [1, BC]],
                                    ),
                                )

                        # xp = x @ W_ih.T + bias, feature-major bf16
                        xp = pxp.tile([128, M, TOK], BF16, tag="xp")
                        for m in range(M):
                            for nn in range(TOK // 512):
                                ps = ppx.tile([128, 512], F32, tag="xps")
                                for k in range(nk):
                                    nc.tensor.matmul(
                                        ps,
                                        wih[:, k, m * 128 : (m + 1) * 128],
                                        xT[:, k, nn * 512 : (nn + 1) * 512],
                                        start=(k == 0),
                                        stop=(k == nk - 1),
                                    )
                                nc.vector.tensor_scalar(
                                    xp[:, m, nn * 512 : (nn + 1) * 512],
                                    ps,
                                    bxp[:, m : m + 1],
                                    None,
                                    op0=ALU.add,
                                )

                        # sequential GRU steps
                        for s in range(TC):
                            t = ci * TC + s
                            gh = pps.tile([128, M, BC], F32, tag="gh")
                            for m in range(M):
                                for k in range(KH):
                                    nc.tensor.matmul(
                                        gh[:, m, :],
                                        whh[:, k, m * 128 : (m + 1) * 128],
                                        h_prev[:, k, :],
                                        start=(k == 0),
                                        stop=(k == KH - 1),
                                    )
                            xps = xp[:, :, s * BC : (s + 1) * BC]
                            rzp = pg.tile([128, 2 * KH, BC], F32, tag="rzp")
                            nc.vector.tensor_tensor(
                                rzp, gh[:, 0 : 2 * KH, :], xps[:, 0 : 2 * KH, :], ALU.add
                            )
                            srz = pg.tile([128, 2 * KH, BC], F32, tag="srz")
                            nc.scalar.activation(srz, rzp, AF.Sigmoid)
                            npre = pg.tile([128, KH, BC], F32, tag="npre")
                            for g in range(KH):
                                nc.vector.scalar_tensor_tensor(
                                    npre[:, g, :],
                                    gh[:, 2 * KH + g, :],
                                    bhn_sb[:, g : g + 1],
                                    srz[:, g, :],
                                    op0=ALU.add,
                                    op1=ALU.mult,
                                )
                            nc.vector.tensor_tensor(
                                npre, npre, xps[:, 2 * KH : 3 * KH, :], ALU.add
                            )
                            nt = pg.tile([128, KH, BC], F32, tag="nt")
                            nc.scalar.activation(nt, npre, AF.Tanh)
                            d = pg.tile([128, KH, BC], F32, tag="d")
                            nc.vector.tensor_tensor(d, h_prev, nt, ALU.subtract)
                            nc.vector.tensor_tensor(d, srz[:, KH : 2 * KH, :], d, ALU.mult)
                            h_new = pst.tile([128, KH, BC], BF16, tag="h")
                            nc.vector.tensor_tensor(h_new, nt, d, ALU.add)
                            nc.sync.dma_start(
                                hseq[t, :, :, :].rearrange("k p b -> p k b"), h_new
                            )
                            h_prev = h_new

                # masked contributions + pair exchange
                contrib = contrib0 if layer == 0 else contrib1
                rs_out = rs0 if layer == 0 else rs1
                with ExitStack() as ctx:
                    pe = ctx.enter_context(tc.tile_pool(name=f"pe{layer}", bufs=3))
                    CH = 16
                    rows = T * KH // CH  # t*k rows per chunk
                    flat_h = hseq[:, :, :, :].rearrange("t k p b -> p (t k) b")
                    flat_c = contrib[:, :, :, :, :].rearrange("r t k p b -> p r (t k) b")
                    for cc in range(CH):
                        hb = pe.tile([128, rows, BC], BF16, tag="hb")
                        nc.sync.dma_start(hb, flat_h[:, cc * rows : (cc + 1) * rows, :])
                        cb = pe.tile([128, 2, rows, BC], BF16, tag="cb")
                        for r in range(2):
                            nc.vector.tensor_scalar(
                                cb[:, r, :, :], hb, mc[:, r : r + 1], None, op0=ALU.mult
                            )
                            nc.sync.dma_start(
                                flat_c[:, r, cc * rows : (cc + 1) * rows, :],
                                cb[:, r, :, :],
                            )
                nc.gpsimd.collective_compute(
                    "ReduceScatter",
                    ALU.add,
                    replica_groups=groups,
                    ins=[contrib[:, :, :, :, :]],
                    outs=[rs_out[:, :, :, :]],
                )

            scan_layer(0)
            scan_layer(1)

            # ================= classifier =================
            TC2 = 32
            TOK2 = TC2 * BC  # 512
            with ExitStack() as ctx:
                pc = ctx.enter_context(tc.tile_pool(name="pc", bufs=2))
                ppc = ctx.enter_context(tc.tile_pool(name="ppc", bufs=2, space="PSUM"))
                st = KH * 128 * BC
                for ci in range(T // TC2):
                    hT = pc.tile([128, K1, TOK2], BF16, tag="hTc")
                    for k in range(KH):
                        nc.sync.dma_start(
                            hT[:, k, :],
                            bass.AP(
                                tensor=h1_own,
                                offset=ci * TC2 * st + k * 128 * BC,
                                ap=[[BC, 128], [st, TC2], [1, BC]],
                            ),
                        )
                        nc.sync.dma_start(
                            hT[:, KH + k, :],
                            bass.AP(
                                tensor=rs1,
                                offset=(T - 1 - ci * TC2) * st + k * 128 * BC,
                                ap=[[BC, 128], [-st, TC2], [1, BC]],
                            ),
                        )
                    ps = ppc.tile([K, TOK2], F32, tag="cls")
                    for k in range(K1):
                        nc.tensor.matmul(
                            ps, cw[:, k, :], hT[:, k, :], start=(k == 0), stop=(k == K1 - 1)
                        )
                    nc.vector.tensor_scalar(
                        emisT[:, ci * TOK2 : (ci + 1) * TOK2],
                        ps,
                        clsb_sb[:, 0:1],
                        None,
                        op0=ALU.add,
                    )
                    nc.sync.dma_start(
                        emis_out[:, ci * TOK2 : (ci + 1) * TOK2],
                        emisT[:, ci * TOK2 : (ci + 1) * TOK2],
                    )
            nc.scalar.activation(EM, emisT, AF.Exp)

            # ================= CRF forward (prob space) =================
            with ExitStack() as ctx:
                pA = ctx.enter_context(tc.tile_pool(name="pA", bufs=3))
                ppA = ctx.enter_context(tc.tile_pool(name="ppA", bufs=2, space="PSUM"))
                A = pA.tile([K, BC], F32, tag="A")
                nc.vector.tensor_scalar(
                    A, EM[:, 0:BC], es_sb[:, 0:1], None, op0=ALU.mult
                )
                nscale = 0
                for t in range(1, T):
                    Sp = ppA.tile([K, BC], F32, tag="S")
                    nc.tensor.matmul(Sp, et_sb, A, start=True, stop=True)
                    An = pA.tile([K, BC], F32, tag="A")
                    nc.vector.tensor_tensor(
                        An, Sp, EM[:, t * BC : (t + 1) * BC], ALU.mult
                    )
                    if t % RESCALE_EVERY == 0:
                        nc.vector.tensor_scalar_mul(An, An, 2.0 ** (-RESCALE_SHIFT))
                        nscale += 1
                    A = An
                nc.vector.tensor_scalar(A, A, ee_sb[:, 0:1], None, op0=ALU.mult)
                den_ps = ppA.tile([1, BC], F32, tag="dps")
                nc.tensor.matmul(den_ps, ones_k, A, start=True, stop=True)
                denom = pA.tile([1, BC], F32, tag="den")
                nc.scalar.activation(denom, den_ps, AF.Ln)
                nc.vector.tensor_scalar_add(
                    denom, denom, float(nscale * RESCALE_SHIFT * math.log(2.0))
                )

                # ---- numerator via one-hots ----
                acc = pA.tile([K, BC], F32, tag="acc")
                nc.vector.memset(acc, 0.0)
                NCH2 = T // TC2
                for ci in range(NCH2):
                    n_tok = TOK2
                    labi = pA.tile([K, TOK2 + BC], I32, tag="labi")
                    n_ext = TOK2 + BC if ci < NCH2 - 1 else TOK2
                    nc.gpsimd.dma_start(
                        labi[:, 0:n_ext],
                        bass.AP(
                            tensor=labs_flat,
                            offset=ci * TOK2,
                            ap=[[0, K], [1, n_ext]],
                        ),
                    )
                    labf = pA.tile([K, TOK2 + BC], F32, tag="labf")
                    nc.vector.tensor_copy(labf[:, 0:n_ext], labi[:, 0:n_ext])
                    # emission term: (lab == k) * emis
                    tmp = pA.tile([K, TOK2], F32, tag="tmp")
                    nc.vector.scalar_tensor_tensor(
                        tmp,
                        labf[:, 0:TOK2],
                        iota_f[:, 0:1],
                        emisT[:, ci * TOK2 : (ci + 1) * TOK2],
                        op0=ALU.is_equal,
                        op1=ALU.mult,
                    )
                    red = pA.tile([K, BC], F32, tag="red")
                    nc.vector.tensor_reduce(
                        red,
                        tmp.rearrange("p (s b) -> p b s", b=BC),
                        axis=mybir.AxisListType.X,
                        op=ALU.add,
                    )
                    nc.vector.tensor_tensor(acc, acc, red, ALU.add)
                    # transition term: onehot(lab_t) @ trans, dot onehot(lab_{t+1})
                    n_pair = TOK2 if ci < NCH2 - 1 else TOK2 - BC
                    oh = pA.tile([K, TOK2], F32, tag="oh")
                    nc.vector.tensor_scalar(
                        oh, labf[:, 0:TOK2], iota_f[:, 0:1], None, op0=ALU.is_equal
                    )
                    trp = ppA.tile([K, TOK2], F32, tag="trp")
                    nc.tensor.matmul(
                        trp[:, 0:n_pair], tm_sb, oh[:, 0:n_pair], start=True, stop=True
                    )
                    tmp2 = pA.tile([K, TOK2], F32, tag="tmp2")
                    nc.vector.scalar_tensor_tensor(
                        tmp2[:, 0:n_pair],
                        labf[:, BC : BC + n_pair],
                        iota_f[:, 0:1],
                        trp[:, 0:n_pair],
                        op0=ALU.is_equal,
                        op1=ALU.mult,
                    )
                    if n_pair < TOK2:
                        nc.vector.memset(tmp2[:, n_pair:TOK2], 0.0)
                    red2 = pA.tile([K, BC], F32, tag="red2")
                    nc.vector.tensor_reduce(
                        red2,
                        tmp2.rearrange("p (s b) -> p b s", b=BC),
                        axis=mybir.AxisListType.X,
                        op=ALU.add,
                    )
                    nc.vector.tensor_tensor(acc, acc, red2, ALU.add)

                # start / end terms
                lab0i = pA.tile([K, BC], I32, tag="lab0i")
                nc.gpsimd.dma_start(
                    lab0i, bass.AP(tensor=labs_flat, offset=0, ap=[[0, K], [1, BC]])
                )
                lab0f = pA.tile([K, BC], F32, tag="lab0f")
                nc.vector.tensor_copy(lab0f, lab0i)
                oh0 = pA.tile([K, BC], F32, tag="oh0")
                nc.vector.tensor_scalar(oh0, lab0f, iota_f[:, 0:1], None, op0=ALU.is_equal)
                nc.vector.tensor_scalar(oh0, oh0, sr_sb[:, 0:1], None, op0=ALU.mult)
                nc.vector.tensor_tensor(acc, acc, oh0, ALU.add)

                labLi = pA.tile([K, BC], I32, tag="labLi")
                nc.gpsimd.dma_start(
                    labLi,
                    bass.AP(tensor=labs_flat, offset=(T - 1) * BC, ap=[[0, K], [1, BC]]),
                )
                labLf = pA.tile([K, BC], F32, tag="labLf")
                nc.vector.tensor_copy(labLf, labLi)
                ohL = pA.tile([K, BC], F32, tag="ohL")
                nc.vector.tensor_scalar(ohL, labLf, iota_f[:, 0:1], None, op0=ALU.is_equal)
                nc.vector.tensor_scalar(ohL, ohL, er_sb[:, 0:1], None, op0=ALU.mult)
                nc.vector.tensor_tensor(acc, acc, ohL, ALU.add)

                num_ps = ppA.tile([1, BC], F32, tag="dps")
                nc.tensor.matmul(num_ps, ones_k, acc, start=True, stop=True)
                ld = pA.tile([1, BC], F32, tag="ld")
                nc.vector.tensor_tensor(ld, denom, num_ps, ALU.subtract)
                lsum = pA.tile([1, 1], F32, tag="lsum")
                nc.vector.tensor_reduce(
                    lsum, ld, axis=mybir.AxisListType.X, op=ALU.add
                )
                nc.sync.dma_start(losspart[:, :], lsum)

    nc.compile()
    return nc


def _prep_inputs(input_ids, labels, emb, weights, T):
    """Build the 8 per-core input maps. weights: dict of reference arrays."""
    import ml_dtypes

    ids = np.asarray(input_ids).astype(np.int32)[:, :T]
    labs = np.asarray(labels).astype(np.int32)[:, :T]
    assert (labs >= 0).all(), "kernel assumes all-true mask (labels >= 0)"
    emb_bf = np.asarray(emb).astype(ml_dtypes.bfloat16)

    def bf(x):
        return np.ascontiguousarray(x).astype(ml_dtypes.bfloat16)

    def f32(x):
        return np.ascontiguousarray(np.asarray(x, np.float32))

    trans = np.asarray(weights["crf_trans"], np.float64)
    start = np.asarray(weights["crf_start"], np.float64)
    end = np.asarray(weights["crf_end"], np.float64)

    in_maps = []
    for c in range(NC_):
        fwd = c < 4
        q = c % 4
        d = "f" if fwd else "b"
        ids_l = ids[q * BC : (q + 1) * BC]
        labs_l = labs[q * BC : (q + 1) * BC]
        if not fwd:
            ids_l = ids_l[:, ::-1]
            labs_l = labs_l[:, ::-1]
        ids_flat = np.ascontiguousarray(ids_l.T).reshape(-1)  # (t, b) b-fast
        labs_flat = np.ascontiguousarray(labs_l.T).reshape(-1)

        wih0 = np.asarray(weights[f"w_ih_l0{d}"], np.float32)  # [3H, E]
        whh0 = np.asarray(weights[f"w_hh_l0{d}"], np.float32)  # [3H, H]
        bih0 = np.asarray(weights[f"b_ih_l0{d}"], np.float32)
        bhh0 = np.asarray(weights[f"b_hh_l0{d}"], np.float32)
        wih1 = np.asarray(weights[f"w_ih_l1{d}"], np.float32)  # [3H, 2H]
        whh1 = np.asarray(weights[f"w_hh_l1{d}"], np.float32)
        bih1 = np.asarray(weights[f"b_ih_l1{d}"], np.float32)
        bhh1 = np.asarray(weights[f"b_hh_l1{d}"], np.float32)

        w_ihT1 = wih1.T  # [2H, 3H], rows: [f-inputs, b-inputs]
        cls_wT = np.asarray(weights["cls_w"], np.float32).T  # [2H, K]
        if not fwd:
            w_ihT1 = np.concatenate([w_ihT1[H:], w_ihT1[:H]], axis=0)
            cls_wT = np.concatenate([cls_wT[H:], cls_wT[:H]], axis=0)

        def mk_bias(bih, bhh):
            bias = bih.copy()
            bias[: 2 * H] += bhh[: 2 * H]
            return bias.reshape(M, 128).astype(np.float32)

        tr = trans if fwd else trans.T
        st = start if fwd else end
        en = end if fwd else start

        in_maps.append(
            {
                "ids_flat": ids_flat,
                "labs_flat": labs_flat,
                "emb": emb_bf,
                "w_ihT0": bf(wih0.T),
                "w_hhT0": bf(whh0.T),
                "w_ihT1": bf(w_ihT1),
                "w_hhT1": bf(whh1.T),
                "bias0": mk_bias(bih0, bhh0),
                "bhn0": bhh0[2 * H :].reshape(KH, 128).astype(np.float32),
                "bias1": mk_bias(bih1, bhh1),
                "bhn1": bhh1[2 * H :].reshape(KH, 128).astype(np.float32),
                "cls_wT": bf(cls_wT),
                "cls_b": f32(weights["cls_b"]),
                "e_trans": np.exp(tr).astype(np.float32),
                "transm": tr.astype(np.float32),
                "start_raw": st.astype(np.float32),
                "end_raw": en.astype(np.float32),
                "exp_start": np.exp(st).astype(np.float32),
                "exp_end": np.exp(en).astype(np.float32),
                "masks": np.array([1.0, 0.0] if not fwd else [0.0, 1.0], np.float32),
            }
        )
    return in_maps


_NC_CACHE = {}


def run_model(input_ids, labels, emb, weights, T=512, TC=32):
    key = (T, TC)
    if key not in _NC_CACHE:
        _NC_CACHE[key] = build_nc(T, TC)
    nc = _NC_CACHE[key]
    in_maps = _prep_inputs(input_ids, labels, emb, weights, T)
    res = run_bass_kernel_spmd(nc, in_maps, core_ids=list(range(NC_))).results
    emissions = np.concatenate(
        [res[c]["emis"].reshape(K, T, BC).transpose(2, 1, 0) for c in range(4)], axis=0
    )
    loss = np.float32(sum(float(res[c]["losspart"][0, 0]) for c in range(4)))
    return loss, emissions


def kernel(
    input_ids,
    labels,
    emb,
    w_ih_l0f, w_hh_l0f, b_ih_l0f, b_hh_l0f,
    w_ih_l0b, w_hh_l0b, b_ih_l0b, b_hh_l0b,
    w_ih_l1f, w_hh_l1f, b_ih_l1f, b_hh_l1f,
    w_ih_l1b, w_hh_l1b, b_ih_l1b, b_hh_l1b,
    cls_w, cls_b, crf_start, crf_end, crf_trans,
):
    weights = dict(
        w_ih_l0f=w_ih_l0f, w_hh_l0f=w_hh_l0f, b_ih_l0f=b_ih_l0f, b_hh_l0f=b_hh_l0f,
        w_ih_l0b=w_ih_l0b, w_hh_l0b=w_hh_l0b, b_ih_l0b=b_ih_l0b, b_hh_l0b=b_hh_l0b,
        w_ih_l1f=w_ih_l1f, w_hh_l1f=w_hh_l1f, b_ih_l1f=b_ih_l1f, b_hh_l1f=b_hh_l1f,
        w_ih_l1b=w_ih_l1b, w_hh_l1b=w_hh_l1b, b_ih_l1b=b_ih_l1b, b_hh_l1b=b_hh_l1b,
        cls_w=cls_w, cls_b=cls_b,
        crf_start=crf_start, crf_end=crf_end, crf_trans=crf_trans,
    )
    loss, emissions = run_model(input_ids, labels, emb, weights, T=512, TC=32)
    return loss, emissions


# revision 8
# speedup vs baseline: 1411.0084x; 1411.0084x over previous
"""BiGRU(2-layer, bidirectional, H=512) + classifier(K=31) + linear-chain CRF
loss on 8 Trainium2 NeuronCores.

Sharding: direction x batch-quarter split. Cores 0-3 run the FORWARD scans for
batch quarters 0-3 (16 batches each); cores 4-7 run the BACKWARD scans for the
same quarters. Time reversal is absorbed entirely into per-core host-prepared
inputs (reversed ids/labels, transposed CRF transitions, swapped start/end,
own-first weight row permutations), so the device program is identical on
every core (SPMD). The layer-boundary f<->b hidden-sequence exchange is a
pair-grouped ReduceScatter with mask-selected contributions. Each pair
redundantly computes classifier+CRF for its quarter; host consumes the
forward cores' outputs.

Scan structure per step (option "weights-stationary"): feature-major hidden
state hT [128p x 4k x 16b] bf16; 48 self-loading bf16 matmuls accumulate
gh = W_hh.T-tiles @ hT into PSUM [128 x 12 x 16]; gates computed feature-major
on DVE/ACT in fp32; h' written back feature-major (no transposes).

CRF forward DP runs in probability space: A_t = (E^T A_{t-1}) * exp(em_t),
with E = exp(trans) precomputed on host, periodic constant rescale by 2^-40
every 8 steps (exactly compensated in the final log), numerator via one-hot
(iota + is_equal) contractions.

Assumes labels >= 0 (mask all-true), which holds for this problem's inputs.
"""

import sys

sys.path.insert(0, "/opt/trn_rl_repo")

import math
from contextlib import ExitStack

import numpy as np

import concourse.bass as bass
import concourse.mybir as mybir
import concourse.tile as tile
from concourse import bacc
from concourse.bass_utils import run_bass_kernel_spmd

AF = mybir.ActivationFunctionType
ALU = mybir.AluOpType
F32 = mybir.dt.float32
BF16 = mybir.dt.bfloat16
I32 = mybir.dt.int32

B, V, E, H, K = 64, 21128, 256, 512, 31
NC_ = 8
BC = 16  # batches per core
M = 12  # gh feature tiles (3H/128)
KH = 4  # hidden k-tiles (H/128)
K0 = 2  # layer-0 input k-tiles (E/128)
K1 = 8  # layer-1 input k-tiles (2H/128)
RESCALE_EVERY = 8
RESCALE_SHIFT = 40.0  # multiply A by 2^-40


def build_nc(T=512, TC=32):
    nc = bacc.Bacc()
    NT = T * BC

    # ---------------- DRAM tensors ----------------
    ids_flat = nc.dram_tensor("ids_flat", [NT], I32, kind="ExternalInput")
    labs_flat = nc.dram_tensor("labs_flat", [NT], I32, kind="ExternalInput")
    emb = nc.dram_tensor("emb", [V, E], BF16, kind="ExternalInput")
    w_ihT0 = nc.dram_tensor("w_ihT0", [E, 3 * H], BF16, kind="ExternalInput")
    w_hhT0 = nc.dram_tensor("w_hhT0", [H, 3 * H], BF16, kind="ExternalInput")
    w_ihT1 = nc.dram_tensor("w_ihT1", [2 * H, 3 * H], BF16, kind="ExternalInput")
    w_hhT1 = nc.dram_tensor("w_hhT1", [H, 3 * H], BF16, kind="ExternalInput")
    bias0 = nc.dram_tensor("bias0", [M, 128], F32, kind="ExternalInput")
    bhn0 = nc.dram_tensor("bhn0", [KH, 128], F32, kind="ExternalInput")
    bias1 = nc.dram_tensor("bias1", [M, 128], F32, kind="ExternalInput")
    bhn1 = nc.dram_tensor("bhn1", [KH, 128], F32, kind="ExternalInput")
    cls_wT = nc.dram_tensor("cls_wT", [2 * H, K], BF16, kind="ExternalInput")
    cls_b = nc.dram_tensor("cls_b", [K], F32, kind="ExternalInput")
    e_trans = nc.dram_tensor("e_trans", [K, K], F32, kind="ExternalInput")
    transm = nc.dram_tensor("transm", [K, K], F32, kind="ExternalInput")
    start_raw = nc.dram_tensor("start_raw", [K], F32, kind="ExternalInput")
    end_raw = nc.dram_tensor("end_raw", [K], F32, kind="ExternalInput")
    exp_start = nc.dram_tensor("exp_start", [K], F32, kind="ExternalInput")
    exp_end = nc.dram_tensor("exp_end", [K], F32, kind="ExternalInput")
    masks = nc.dram_tensor("masks", [2], F32, kind="ExternalInput")

    emis_out = nc.dram_tensor("emis", [K, T * BC], F32, kind="ExternalOutput")
    losspart = nc.dram_tensor("losspart", [1, 1], F32, kind="ExternalOutput")

    h0_own = nc.dram_tensor("h0_own", [T, KH, 128, BC], BF16)
    h1_own = nc.dram_tensor("h1_own", [T, KH, 128, BC], BF16)
    contrib0 = nc.dram_tensor("contrib0", [2, T, KH, 128, BC], BF16)
    contrib1 = nc.dram_tensor("contrib1", [2, T, KH, 128, BC], BF16)
    rs0 = nc.dram_tensor("rs0", [T, KH, 128, BC], BF16)
    rs1 = nc.dram_tensor("rs1", [T, KH, 128, BC], BF16)

    NCH = T // TC  # scan chunks
    TOK = TC * BC  # tokens per scan chunk
    groups = [[q, 4 + q] for q in range(4)]

    with tile.TileContext(nc) as tc:
        with ExitStack() as ctx0:
            pw = ctx0.enter_context(tc.tile_pool(name="pw", bufs=1))

            # ---- persistent loads ----
            w0 = pw.tile([128, K0, 3 * H], BF16)
            nc.sync.dma_start(w0, w_ihT0[:, :].rearrange("(k p) n -> p k n", p=128))
            wh0 = pw.tile([128, KH, 3 * H], BF16)
            nc.sync.dma_start(wh0, w_hhT0[:, :].rearrange("(k p) n -> p k n", p=128))
            w1 = pw.tile([128, K1, 3 * H], BF16)
            nc.sync.dma_start(w1, w_ihT1[:, :].rearrange("(k p) n -> p k n", p=128))
            wh1 = pw.tile([128, KH, 3 * H], BF16)
            nc.sync.dma_start(wh1, w_hhT1[:, :].rearrange("(k p) n -> p k n", p=128))
            cw = pw.tile([128, K1, K], BF16)
            nc.sync.dma_start(cw, cls_wT[:, :].rearrange("(k p) n -> p k n", p=128))
            b0_sb = pw.tile([128, M], F32)
            nc.sync.dma_start(b0_sb, bias0[:, :].rearrange("m p -> p m"))
            bh0_sb = pw.tile([128, KH], F32)
            nc.sync.dma_start(bh0_sb, bhn0[:, :].rearrange("m p -> p m"))
            b1_sb = pw.tile([128, M], F32)
            nc.sync.dma_start(b1_sb, bias1[:, :].rearrange("m p -> p m"))
            bh1_sb = pw.tile([128, KH], F32)
            nc.sync.dma_start(bh1_sb, bhn1[:, :].rearrange("m p -> p m"))
            clsb_sb = pw.tile([K, 1], F32)
            nc.sync.dma_start(clsb_sb, cls_b[:, None])
            et_sb = pw.tile([K, K], F32)
            nc.sync.dma_start(et_sb, e_trans[:, :])
            tm_sb = pw.tile([K, K], F32)
            nc.sync.dma_start(tm_sb, transm[:, :])
            sr_sb = pw.tile([K, 1], F32)
            nc.sync.dma_start(sr_sb, start_raw[:, None])
            er_sb = pw.tile([K, 1], F32)
            nc.sync.dma_start(er_sb, end_raw[:, None])
            es_sb = pw.tile([K, 1], F32)
            nc.sync.dma_start(es_sb, exp_start[:, None])
            ee_sb = pw.tile([K, 1], F32)
            nc.sync.dma_start(ee_sb, exp_end[:, None])
            mc = pw.tile([128, 2], F32)
            nc.gpsimd.dma_start(mc, bass.AP(tensor=masks, offset=0, ap=[[0, 128], [1, 2]]))
            iota_k = pw.tile([K, 1], I32)
            nc.gpsimd.iota(iota_k, [[0, 1]], channel_multiplier=1)
            iota_f = pw.tile([K, 1], F32)
            nc.vector.tensor_copy(iota_f, iota_k)
            zero_h = pw.tile([128, KH, BC], BF16)
            nc.vector.memset(zero_h, 0.0)
            ones_k = pw.tile([K, 1], F32)
            nc.vector.memset(ones_k, 1.0)
            emisT = pw.tile([K, NT], F32)
            EM = pw.tile([K, NT], F32)

            # ================= scan layers =================
            def scan_layer(layer):
                wih = w0 if layer == 0 else w1
                whh = wh0 if layer == 0 else wh1
                bxp = b0_sb if layer == 0 else b1_sb
                bhn_sb = bh0_sb if layer == 0 else bh1_sb
                nk = K0 if layer == 0 else K1
                hseq = h0_own if layer == 0 else h1_own

                with ExitStack() as ctx:
                    px = ctx.enter_context(tc.tile_pool(name=f"px{layer}", bufs=2))
                    pxp = ctx.enter_context(tc.tile_pool(name=f"pxp{layer}", bufs=2))
                    pst = ctx.enter_context(tc.tile_pool(name=f"pst{layer}", bufs=3))
                    pg = ctx.enter_context(tc.tile_pool(name=f"pg{layer}", bufs=3))
                    pps = ctx.enter_context(
                        tc.tile_pool(name=f"pps{layer}", bufs=1, space="PSUM")
                    )
                    ppx = ctx.enter_context(
                        tc.tile_pool(name=f"ppx{layer}", bufs=2, space="PSUM")
                    )

                    h_prev = zero_h
                    for ci in range(NCH):
                        xT = px.tile([128, nk, TOK], BF16, tag="xT")
                        if layer == 0:
                            # gather embedding rows + transpose to feature-major
                            for g in range(TOK // 128):
                                ids_sb = pg.tile([128, 1], I32, tag="ids")
                                nc.gpsimd.dma_start(
                                    ids_sb,
                                    ids_flat[ci * TOK + g * 128 : ci * TOK + (g + 1) * 128][
                                        :, None
                                    ],
                                )
                                xg = pg.tile([128, E], BF16, tag="xg")
                                nc.gpsimd.indirect_dma_start(
                                    out=xg,
                                    out_offset=None,
                                    in_=emb[:, :],
                                    in_offset=bass.IndirectOffsetOnAxis(
                                        ap=ids_sb[:, :1], axis=0
                                    ),
                                )
                                for k in range(K0):
                                    nc.sync.dma_start_transpose(
                                        xT[:, k, g * 128 : (g + 1) * 128],
                                        xg[:, k * 128 : (k + 1) * 128],
                                    )
                        else:
                            # own half ascending, partner half reversed local order
                            st = KH * 128 * BC  # t-stride in elements
                            for k in range(KH):
                                nc.sync.dma_start(
                                    xT[:, k, :],
                                    bass.AP(
                                        tensor=h0_own,
                                        offset=ci * TC * st + k * 128 * BC,
                                        ap=[[BC, 128], [st, TC], [1, BC]],
                                    ),
                                )
                                nc.sync.dma_start(
                                    xT[:, KH + k, :],
                                    bass.AP(
                                        tensor=rs0,
                                        offset=(T - 1 - ci * TC) * st + k * 128 * BC,
                                        ap=[[BC, 128], [-st, TC], [1, BC]],
                                    ),
                                )

                        # xp = x @ W_ih.T + bias, feature-major bf16
                        xp = pxp.tile([128, M, TOK], BF16, tag="xp")
                        for m in range(M):
                            for nn in range(TOK // 512):
                                ps = ppx.tile([128, 512], F32, tag="xps")
                                for k in range(nk):
                                    nc.tensor.matmul(
                                        ps,
                                        wih[:, k, m * 128 : (m + 1) * 128],
                                        xT[:, k, nn * 512 : (nn + 1) * 512],
                                        start=(k == 0),
                                        stop=(k == nk - 1),
                                    )
                                nc.vector.tensor_scalar(
                                    xp[:, m, nn * 512 : (nn + 1) * 512],
                                    ps,
                                    bxp[:, m : m + 1],
                                    None,
                                    op0=ALU.add,
                                )

                        # sequential GRU steps, 4 independent per-block chains:
                        # psum tile j holds m-tiles {j (r), j+4 (z), j+8 (n)}
                        # so h-block j's gates close locally and next step's
                        # k=j matmuls pipeline with other blocks' gate chains.
                        for s in range(TC):
                            t = ci * TC + s
                            ghs = []
                            for j in range(KH):
                                ghj = pps.tile([128, 3, BC], F32, tag=f"gh{j}")
                                ghs.append(ghj)
                            for j in range(KH):
                                for r3, m in enumerate((j, j + KH, j + 2 * KH)):
                                    for k in range(KH):
                                        nc.tensor.matmul(
                                            ghs[j][:, r3, :],
                                            whh[:, k, m * 128 : (m + 1) * 128],
                                            h_prev[:, k, :],
                                            start=(k == 0),
                                            stop=(k == KH - 1),
                                        )
                            h_new = pst.tile([128, KH, BC], BF16, tag="h")
                            for j in range(KH):
                                xps_rz = xp[
                                    :, j : j + KH + 1 : KH, s * BC : (s + 1) * BC
                                ]
                                rzp = pg.tile([128, 2, BC], F32, tag=f"rzp{j}")
                                nc.vector.tensor_tensor(
                                    rzp, ghs[j][:, 0:2, :], xps_rz, ALU.add
                                )
                                srz = pg.tile([128, 2, BC], F32, tag=f"srz{j}")
                                nc.scalar.activation(srz, rzp, AF.Sigmoid)
                                npre = pg.tile([128, BC], F32, tag=f"npre{j}")
                                nc.vector.scalar_tensor_tensor(
                                    npre,
                                    ghs[j][:, 2, :],
                                    bhn_sb[:, j : j + 1],
                                    srz[:, 0, :],
                                    op0=ALU.add,
                                    op1=ALU.mult,
                                )
                                nc.vector.tensor_tensor(
                                    npre,
                                    npre,
                                    xp[:, 2 * KH + j, s * BC : (s + 1) * BC],
                                    ALU.add,
                                )
                                nt = pg.tile([128, BC], F32, tag=f"nt{j}")
                                nc.scalar.activation(nt, npre, AF.Tanh)
                                d = pg.tile([128, BC], F32, tag=f"d{j}")
                                nc.vector.tensor_tensor(
                                    d, h_prev[:, j, :], nt, ALU.subtract
                                )
                                nc.vector.tensor_tensor(d, srz[:, 1, :], d, ALU.mult)
                                nc.vector.tensor_tensor(h_new[:, j, :], nt, d, ALU.add)
                            nc.sync.dma_start(
                                hseq[t, :, :, :].rearrange("k p b -> p k b"), h_new
                            )
                            h_prev = h_new

                # masked contributions + pair exchange
                contrib = contrib0 if layer == 0 else contrib1
                rs_out = rs0 if layer == 0 else rs1
                with ExitStack() as ctx:
                    pe = ctx.enter_context(tc.tile_pool(name=f"pe{layer}", bufs=3))
                    CH = 16
                    rows = T * KH // CH  # t*k rows per chunk
                    flat_h = hseq[:, :, :, :].rearrange("t k p b -> p (t k) b")
                    flat_c = contrib[:, :, :, :, :].rearrange("r t k p b -> p r (t k) b")
                    for cc in range(CH):
                        hb = pe.tile([128, rows, BC], BF16, tag="hb")
                        nc.sync.dma_start(hb, flat_h[:, cc * rows : (cc + 1) * rows, :])
                        cb = pe.tile([128, 2, rows, BC], BF16, tag="cb")
                        for r in range(2):
                            nc.vector.tensor_scalar(
                                cb[:, r, :, :], hb, mc[:, r : r + 1], None, op0=ALU.mult
                            )
                            nc.sync.dma_start(
                                flat_c[:, r, cc * rows : (cc + 1) * rows, :],
                                cb[:, r, :, :],
                            )
                nc.gpsimd.collective_compute(
                    "ReduceScatter",
                    ALU.add,
                    replica_groups=groups,
                    ins=[contrib[:, :, :, :, :]],
                    outs=[rs_out[:, :, :, :]],
                )

            scan_layer(0)
            scan_layer(1)

            # ================= classifier =================
            TC2 = 32
            TOK2 = TC2 * BC  # 512
            with ExitStack() as ctx:
                pc = ctx.enter_context(tc.tile_pool(name="pc", bufs=2))
                ppc = ctx.enter_context(tc.tile_pool(name="ppc", bufs=2, space="PSUM"))
                st = KH * 128 * BC
                for ci in range(T // TC2):
                    hT = pc.tile([128, K1, TOK2], BF16, tag="hTc")
                    for k in range(KH):
                        nc.sync.dma_start(
                            hT[:, k, :],
                            bass.AP(
                                tensor=h1_own,
                                offset=ci * TC2 * st + k * 128 * BC,
                                ap=[[BC, 128], [st, TC2], [1, BC]],
                            ),
                        )
                        nc.sync.dma_start(
                            hT[:, KH + k, :],
                            bass.AP(
                                tensor=rs1,
                                offset=(T - 1 - ci * TC2) * st + k * 128 * BC,
                                ap=[[BC, 128], [-st, TC2], [1, BC]],
                            ),
                        )
                    ps = ppc.tile([K, TOK2], F32, tag="cls")
                    for k in range(K1):
                        nc.tensor.matmul(
                            ps, cw[:, k, :], hT[:, k, :], start=(k == 0), stop=(k == K1 - 1)
                        )
                    nc.vector.tensor_scalar(
                        emisT[:, ci * TOK2 : (ci + 1) * TOK2],
                        ps,
                        clsb_sb[:, 0:1],
                        None,
                        op0=ALU.add,
                    )
                    nc.sync.dma_start(
                        emis_out[:, ci * TOK2 : (ci + 1) * TOK2],
                        emisT[:, ci * TOK2 : (ci + 1) * TOK2],
                    )
            nc.scalar.activation(EM, emisT, AF.Exp)

            # ================= CRF forward (prob space) =================
            with ExitStack() as ctx:
                pA = ctx.enter_context(tc.tile_pool(name="pA", bufs=3))
                ppA = ctx.enter_context(tc.tile_pool(name="ppA", bufs=2, space="PSUM"))
                A = pA.tile([K, BC], F32, tag="A")
                nc.vector.tensor_scalar(
                    A, EM[:, 0:BC], es_sb[:, 0:1], None, op0=ALU.mult
                )
                nscale = 0
                for t in range(1, T):
                    Sp = ppA.tile([K, BC], F32, tag="S")
                    nc.tensor.matmul(Sp, et_sb, A, start=True, stop=True)
                    An = pA.tile([K, BC], F32, tag="A")
                    nc.vector.tensor_tensor(
                        An, Sp, EM[:, t * BC : (t + 1) * BC], ALU.mult
                    )
                    if t % RESCALE_EVERY == 0:
                        nc.vector.tensor_scalar_mul(An, An, 2.0 ** (-RESCALE_SHIFT))
                        nscale += 1
                    A = An
                nc.vector.tensor_scalar(A, A, ee_sb[:, 0:1], None, op0=ALU.mult)
                den_ps = ppA.tile([1, BC], F32, tag="dps")
                nc.tensor.matmul(den_ps, ones_k, A, start=True, stop=True)
                denom = pA.tile([1, BC], F32, tag="den")
                nc.scalar.activation(denom, den_ps, AF.Ln)
                nc.vector.tensor_scalar_add(
                    denom, denom, float(nscale * RESCALE_SHIFT * math.log(2.0))
                )

                # ---- numerator via one-hots ----
                acc = pA.tile([K, BC], F32, tag="acc")
                nc.vector.memset(acc, 0.0)
                NCH2 = T // TC2
                for ci in range(NCH2):
                    n_tok = TOK2
                    labi = pA.tile([K, TOK2 + BC], I32, tag="labi")
                    n_ext = TOK2 + BC if ci < NCH2 - 1 else TOK2
                    nc.gpsimd.dma_start(
                        labi[:, 0:n_ext],
                        bass.AP(
                            tensor=labs_flat,
                            offset=ci * TOK2,
                            ap=[[0, K], [1, n_ext]],
                        ),
                    )
                    labf = pA.tile([K, TOK2 + BC], F32, tag="labf")
                    nc.vector.tensor_copy(labf[:, 0:n_ext], labi[:, 0:n_ext])
                    # emission term: (lab == k) * emis
                    tmp = pA.tile([K, TOK2], F32, tag="tmp")
                    nc.vector.scalar_tensor_tensor(
                        tmp,
                        labf[:, 0:TOK2],
                        iota_f[:, 0:1],
                        emisT[:, ci * TOK2 : (ci + 1) * TOK2],
                        op0=ALU.is_equal,
                        op1=ALU.mult,
                    )
                    red = pA.tile([K, BC], F32, tag="red")
                    nc.vector.tensor_reduce(
                        red,
                        tmp.rearrange("p (s b) -> p b s", b=BC),
                        axis=mybir.AxisListType.X,
                        op=ALU.add,
                    )
                    nc.vector.tensor_tensor(acc, acc, red, ALU.add)
                    # transition term: onehot(lab_t) @ trans, dot onehot(lab_{t+1})
                    n_pair = TOK2 if ci < NCH2 - 1 else TOK2 - BC
                    oh = pA.tile([K, TOK2], F32, tag="oh")
                    nc.vector.tensor_scalar(
                        oh, labf[:, 0:TOK2], iota_f[:, 0:1], None, op0=ALU.is_equal
                    )
                    trp = ppA.tile([K, TOK2], F32, tag="trp")
                    nc.tensor.matmul(
                        trp[:, 0:n_pair], tm_sb, oh[:, 0:n_pair], start=True, stop=True
                    )
                    tmp2 = pA.tile([K, TOK2], F32, tag="tmp2")
                    nc.vector.scalar_tensor_tensor(
                        tmp2[:, 0:n_pair],
                        labf[:, BC : BC + n_pair],
                        iota_f[:, 0:1],
                        trp[:, 0:n_pair],
                        op0=ALU.is_equal,
                        op1=ALU.mult,
                    )
                    if n_pair < TOK2:
                        nc.vector.memset(tmp2[:, n_pair:TOK2], 0.0)
                    red2 = pA.tile([K, BC], F32, tag="red2")
                    nc.vector.tensor_reduce(
                        red2,
                        tmp2.rearrange("p (s b) -> p b s", b=BC),
                        axis=mybir.AxisListType.X,
                        op=ALU.add,
                    )
                    nc.vector.tensor_tensor(acc, acc, red2, ALU.add)

                # start / end terms
                lab0i = pA.tile([K, BC], I32, tag="lab0i")
                nc.gpsimd.dma_start(
                    lab0i, bass.AP(tensor=labs_flat, offset=0, ap=[[0, K], [1, BC]])
                )
                lab0f = pA.tile([K, BC], F32, tag="lab0f")
                nc.vector.tensor_copy(lab0f, lab0i)
                oh0 = pA.tile([K, BC], F32, tag="oh0")
                nc.vector.tensor_scalar(oh0, lab0f, iota_f[:, 0:1], None, op0=ALU.is_equal)
                nc.vector.tensor_scalar(oh0, oh0, sr_sb[:, 0:1], None, op0=ALU.mult)
                nc.vector.tensor_tensor(acc, acc, oh0, ALU.add)

                labLi = pA.tile([K, BC], I32, tag="labLi")
                nc.gpsimd.dma_start(
                    labLi,
                    bass.AP(tensor=labs_flat, offset=(T - 1) * BC, ap=[[0, K], [1, BC]]),
                )
                labLf = pA.tile([K, BC], F32, tag="labLf")
                nc.vector.tensor_copy(labLf, labLi)
                ohL = pA.tile([K, BC], F32, tag="ohL")
                nc.vector.tensor_scalar(ohL, labLf, iota_f[:, 0:1], None, op0=ALU.is_equal)
                nc.vector.tensor_scalar(ohL, ohL, er_sb[:, 0:1], None, op0=ALU.mult)
                nc.vector.tensor_tensor(acc, acc, ohL, ALU.add)

                num_ps = ppA.tile([1, BC], F32, tag="dps")
                nc.tensor.matmul(num_ps, ones_k, acc, start=True, stop=True)
                ld = pA.tile([1, BC], F32, tag="ld")
                nc.vector.tensor_tensor(ld, denom, num_ps, ALU.subtract)
                lsum = pA.tile([1, 1], F32, tag="lsum")
                nc.vector.tensor_reduce(
                    lsum, ld, axis=mybir.AxisListType.X, op=ALU.add
                )
                nc.sync.dma_start(losspart[:, :], lsum)

    nc.compile()
    return nc


def _prep_inputs(input_ids, labels, emb, weights, T):
    """Build the 8 per-core input maps. weights: dict of reference arrays."""
    import ml_dtypes

    ids = np.asarray(input_ids).astype(np.int32)[:, :T]
    labs = np.asarray(labels).astype(np.int32)[:, :T]
    assert (labs >= 0).all(), "kernel assumes all-true mask (labels >= 0)"
    emb_bf = np.asarray(emb).astype(ml_dtypes.bfloat16)

    def bf(x):
        return np.ascontiguousarray(x).astype(ml_dtypes.bfloat16)

    def f32(x):
        return np.ascontiguousarray(np.asarray(x, np.float32))

    trans = np.asarray(weights["crf_trans"], np.float64)
    start = np.asarray(weights["crf_start"], np.float64)
    end = np.asarray(weights["crf_end"], np.float64)

    in_maps = []
    for c in range(NC_):
        fwd = c < 4
        q = c % 4
        d = "f" if fwd else "b"
        ids_l = ids[q * BC : (q + 1) * BC]
        labs_l = labs[q * BC : (q + 1) * BC]
        if not fwd:
            ids_l = ids_l[:, ::-1]
            labs_l = labs_l[:, ::-1]
        ids_flat = np.ascontiguousarray(ids_l.T).reshape(-1)  # (t, b) b-fast
        labs_flat = np.ascontiguousarray(labs_l.T).reshape(-1)

        wih0 = np.asarray(weights[f"w_ih_l0{d}"], np.float32)  # [3H, E]
        whh0 = np.asarray(weights[f"w_hh_l0{d}"], np.float32)  # [3H, H]
        bih0 = np.asarray(weights[f"b_ih_l0{d}"], np.float32)
        bhh0 = np.asarray(weights[f"b_hh_l0{d}"], np.float32)
        wih1 = np.asarray(weights[f"w_ih_l1{d}"], np.float32)  # [3H, 2H]
        whh1 = np.asarray(weights[f"w_hh_l1{d}"], np.float32)
        bih1 = np.asarray(weights[f"b_ih_l1{d}"], np.float32)
        bhh1 = np.asarray(weights[f"b_hh_l1{d}"], np.float32)

        w_ihT1 = wih1.T  # [2H, 3H], rows: [f-inputs, b-inputs]
        cls_wT = np.asarray(weights["cls_w"], np.float32).T  # [2H, K]
        if not fwd:
            w_ihT1 = np.concatenate([w_ihT1[H:], w_ihT1[:H]], axis=0)
            cls_wT = np.concatenate([cls_wT[H:], cls_wT[:H]], axis=0)

        def mk_bias(bih, bhh):
            bias = bih.copy()
            bias[: 2 * H] += bhh[: 2 * H]
            return bias.reshape(M, 128).astype(np.float32)

        tr = trans if fwd else trans.T
        st = start if fwd else end
        en = end if fwd else start

        in_maps.append(
            {
                "ids_flat": ids_flat,
                "labs_flat": labs_flat,
                "emb": emb_bf,
                "w_ihT0": bf(wih0.T),
                "w_hhT0": bf(whh0.T),
                "w_ihT1": bf(w_ihT1),
                "w_hhT1": bf(whh1.T),
                "bias0": mk_bias(bih0, bhh0),
                "bhn0": bhh0[2 * H :].reshape(KH, 128).astype(np.float32),
                "bias1": mk_bias(bih1, bhh1),
                "bhn1": bhh1[2 * H :].reshape(KH, 128).astype(np.float32),
                "cls_wT": bf(cls_wT),
                "cls_b": f32(weights["cls_b"]),
                "e_trans": np.exp(tr).astype(np.float32),
                "transm": tr.astype(np.float32),
                "start_raw": st.astype(np.float32),
                "end_raw": en.astype(np.float32),
                "exp_start": np.exp(st).astype(np.float32),
                "exp_end": np.exp(en).astype(np.float32),
                "masks": np.array([1.0, 0.0] if not fwd else [0.0, 1.0], np.float32),
            }
        )
    return in_maps


_NC_CACHE = {}


def run_model(input_ids, labels, emb, weights, T=512, TC=32):
    key = (T, TC)
    if key not in _NC_CACHE:
        _NC_CACHE[key] = build_nc(T, TC)
    nc = _NC_CACHE[key]
    in_maps = _prep_inputs(input_ids, labels, emb, weights, T)
    res = run_bass_kernel_spmd(nc, in_maps, core_ids=list(range(NC_))).results
    emissions = np.concatenate(
        [res[c]["emis"].reshape(K, T, BC).transpose(2, 1, 0) for c in range(4)], axis=0
    )
    loss = np.float32(sum(float(res[c]["losspart"][0, 0]) for c in range(4)))
    return loss, emissions


def kernel(
    input_ids,
    labels,
    emb,
    w_ih_l0f, w_hh_l0f, b_ih_l0f, b_hh_l0f,
    w_ih_l0b, w_hh_l0b, b_ih_l0b, b_hh_l0b,
    w_ih_l1f, w_hh_l1f, b_ih_l1f, b_hh_l1f,
    w_ih_l1b, w_hh_l1b, b_ih_l1b, b_hh_l1b,
    cls_w, cls_b, crf_start, crf_end, crf_trans,
):
    weights = dict(
        w_ih_l0f=w_ih_l0f, w_hh_l0f=w_hh_l0f, b_ih_l0f=b_ih_l0f, b_hh_l0f=b_hh_l0f,
        w_ih_l0b=w_ih_l0b, w_hh_l0b=w_hh_l0b, b_ih_l0b=b_ih_l0b, b_hh_l0b=b_hh_l0b,
        w_ih_l1f=w_ih_l1f, w_hh_l1f=w_hh_l1f, b_ih_l1f=b_ih_l1f, b_hh_l1f=b_hh_l1f,
        w_ih_l1b=w_ih_l1b, w_hh_l1b=w_hh_l1b, b_ih_l1b=b_ih_l1b, b_hh_l1b=b_hh_l1b,
        cls_w=cls_w, cls_b=cls_b,
        crf_start=crf_start, crf_end=crf_end, crf_trans=crf_trans,
    )
    loss, emissions = run_model(input_ids, labels, emb, weights, T=512, TC=32)
    return loss, emissions


def build_nop(T=512):
    """Same I/O signature as build_nc but no compute — for calibrating the
    axon RPC + transfer overhead so device time can be estimated by diff."""
    nc = bacc.Bacc()
    NT = T * BC
    for name, shape, dt in [
        ("ids_flat", [NT], I32), ("labs_flat", [NT], I32), ("emb", [V, E], BF16),
        ("w_ihT0", [E, 3 * H], BF16), ("w_hhT0", [H, 3 * H], BF16),
        ("w_ihT1", [2 * H, 3 * H], BF16), ("w_hhT1", [H, 3 * H], BF16),
        ("bias0", [M, 128], F32), ("bhn0", [KH, 128], F32),
        ("bias1", [M, 128], F32), ("bhn1", [KH, 128], F32),
        ("cls_wT", [2 * H, K], BF16), ("cls_b", [K], F32),
        ("e_trans", [K, K], F32), ("transm", [K, K], F32),
        ("start_raw", [K], F32), ("end_raw", [K], F32),
        ("exp_start", [K], F32), ("exp_end", [K], F32), ("masks", [2], F32),
    ]:
        nc.dram_tensor(name, shape, dt, kind="ExternalInput")
    emis_out = nc.dram_tensor("emis", [K, NT], F32, kind="ExternalOutput")
    losspart = nc.dram_tensor("losspart", [1, 1], F32, kind="ExternalOutput")
    from contextlib import ExitStack as _ES
    with tile.TileContext(nc) as tc:
        with _ES() as ctx:
            p = ctx.enter_context(tc.tile_pool(name="p", bufs=1))
            z = p.tile([K, NT], F32)
            nc.vector.memset(z, 0.0)
            nc.sync.dma_start(emis_out[:, :], z)
            z2 = p.tile([1, 1], F32)
            nc.vector.memset(z2, 0.0)
            nc.sync.dma_start(losspart[:, :], z2)
    nc.compile()
    return nc


def run_nop(input_ids, labels, emb, weights, T=512):
    key = ("nop", T)
    if key not in _NC_CACHE:
        _NC_CACHE[key] = build_nop(T)
    nc = _NC_CACHE[key]
    in_maps = _prep_inputs(input_ids, labels, emb, weights, T)
    run_bass_kernel_spmd(nc, in_maps, core_ids=list(range(NC_)))
